# revision 45
# baseline (speedup 1.0000x reference)
"""AttentionClustering (vq_codebook) Trainium2 kernel, 8-core data parallel.

Shard: 8 cores = 4 images x 2 half-images (128 output rows each). Odd cores
get a vertically flipped shard + row-flipped conv weights so every core's
program is identical (true image edge at local top, interior halo at bottom).

Math: q1 = relu(conv3x3(x, w1) + b1); q2 = relu(conv3x3(q1, w2) + b2)  (both
with replicate padding); then the 1x1 conv + cluster-distance softmax folds to
  logit[px, k] = sum_ci q2[ci, px] * muW[k, ci] + bp[k]
  muW = 2 * mu @ W3,  bp = 2 * mu @ b3 - |mu|^2      (|q|^2 cancels in softmax)
  out[px] = sum_k softmax_k(logit) * label[k]

conv1 is direct (9 taps x 64ch = 4.5 in-tiles -> 5 matmuls per 2-row group,
the 64-wide tap-(2,2) weight zero-padded to 128 partitions to avoid the PE
half-array mode). conv2 runs as 1D-Winograd F(2,3) along rows: 4 position
GEMMs of 768 in-dims per 2 output rows instead of direct 2304 (-33% PE
cycles); forward transform t = B^T d on vector+gpsimd in fp16, inverse
y = A^T Y on vector in f32 (Y1 staged via a scalar copy since vector ops
read at most one PSUM operand), relu+bias on scalar. Two row-groups are
processed per pass so every matmul streams 512 columns - at 256 the PE
sequencer is bound by LDWEIGHTS+MATMUL dispatch (~126ns vs the 107ns
stream). Logits for pair N-1 issue after pair N's GEMMs so the PE never
waits on the inverse/relu chain; both groups share one psum tile and one
fused softmax. Output is PE-transposed to row-major so the store is 32
contiguous 1KB rows (a 4-byte-element scatter cost ~24us exposed at the
tail). All fp16 matmuls, f32 psum/softmax; rel err vs f32 reference 1.6e-3.

PE measures ~95% busy at the fp16 roofline for this decomposition; HW exec
~515us on 8 cores at the 2.4GHz PE clock (~640us when the part throttles
to ~2.0GHz, which this box does on some runs).
"""
import sys
if '/opt/trn_rl_repo' not in sys.path:
    sys.path.insert(0, '/opt/trn_rl_repo')

import numpy as np
import concourse.bass as bass
import concourse.mybir as mybir
from concourse import bacc, tile
from concourse.bass_utils import run_bass_kernel_spmd

F32 = mybir.dt.float32
F16 = mybir.dt.float16
AF = mybir.ActivationFunctionType
ALU = mybir.AluOpType
AX = mybir.AxisListType

B, CIN, H, W = 4, 64, 256, 256
Q, K = 256, 16
RB = 32           # output rows per band
NBAND = 4         # bands per core (128 rows)
NCORES = 8

_cached = {}


def build_nc():
    nc = bacc.Bacc("TRN2", target_bir_lowering=False, debug=False)

    CHS = 132 * (W + 2)          # per-channel element stride in flat xh
    xh = nc.declare_dram_parameter("xh", [CIN * CHS + 2 * (W + 2)], F16,
                                   isOutput=False)
    w1a = nc.declare_dram_parameter("w1a", [128, 6, 128], F16, isOutput=False)
    w1r = nc.declare_dram_parameter("w1r", [128, 2, 128], F16, isOutput=False)
    w1s = nc.declare_dram_parameter("w1s", [128, 2, 128], F16, isOutput=False)
    w2l = nc.declare_dram_parameter("w2l", [128, 48, 128], F16, isOutput=False)
    muw = nc.declare_dram_parameter("muw", [128, 2, K], F16, isOutput=False)
    cst = nc.declare_dram_parameter("cst", [128, 2 * K + 4], F32, isOutput=False)
    idn = nc.declare_dram_parameter("idn", [128, 128], F32, isOutput=False)
    outd = nc.declare_dram_parameter("out", [128, W], F32, isOutput=True)

    with tile.TileContext(nc) as tc:
        with tc.tile_pool(name="singles", bufs=1) as singles, \
             tc.tile_pool(name="xpool", bufs=2) as xpool, \
             tc.tile_pool(name="q1pool", bufs=2) as q1pool, \
             tc.tile_pool(name="q2pool", bufs=2) as q2pool, \
             tc.tile_pool(name="tpool", bufs=2) as tpool, \
             tc.tile_pool(name="smx", bufs=2) as smx, \
             tc.tile_pool(name="obuf", bufs=2) as obuf, \
             tc.tile_pool(name="ps", bufs=2, space="PSUM") as ps:
            # Single PSUM pool: two tags of [128,2,2,256] f32 (2 banks each,
            # 2 bufs) = all 8 banks. conv1 chains, conv2 position pairs,
            # logits and the output transpose all rotate through these.

            def ps_tile(tag, name):
                return ps.tile([128, 2, 2, W], F32, tag=tag, name=name)

            # w1a loads first (small); its landing also unblocks the PE
            # warmup matmuls (junk math into a recycled psum slot) that keep
            # the HAM clock-gate ramping through the initial x DMA wait.
            w1abuf = singles.tile([128, 6, 128], F16, tag="w1abuf")
            nc.sync.dma_start(out=w1abuf, in_=w1a.ap())
            w1a_sb = {(mc, dr): w1abuf[:, mc * 3 + dr, :]
                      for mc in range(2) for dr in range(3)}
            wmv = w1abuf.rearrange("p a b -> p (a b)")[:, 0:512] \
                .rearrange("p (a b) -> p a b", a=2)
            for _ in range(22):
                wps = ps_tile("pp_a", "wps")
                nc.tensor.matmul(wps[:, 0], w1abuf[:, 0, :], wmv,
                                 start=True, stop=True)

            xh_ap = xh.ap()

            def xsrc(r0, lo, hi, shift):
                # [64ch, (rows cols) flat] slice of xh, shifted by `shift`
                # elements (1 = one column, W+2 = one row). Rows within a
                # channel are contiguous, so flattening (rows, cols) into one
                # dim gives one big descriptor per channel instead of one
                # per row (10x fewer; the startup chunk went from ~28us to
                # ~4us of DMA).
                return bass.AP(
                    tensor=xh_ap.tensor,
                    offset=(r0 + lo) * (W + 2) + shift,
                    ap=[[CHS, CIN], [1, (hi - lo) * (W + 2)]])

            def emit_xchunks(xa, xr, r0, chunks):
                xaf = xa.rearrange("p r c -> p (r c)")
                xrf = xr.rearrange("p r c -> p (r c)")
                for lo, hi in chunks:
                    s = slice(lo * (W + 2), hi * (W + 2))
                    nc.sync.dma_start(out=xaf[0:64, s],
                                      in_=xsrc(r0, lo, hi, 0))
                    nc.sync.dma_start(out=xaf[64:128, s],
                                      in_=xsrc(r0, lo, hi, 1))
                    nc.sync.dma_start(out=xrf[0:64, s],
                                      in_=xsrc(r0, lo, hi, 0))
                    nc.sync.dma_start(out=xrf[64:128, s],
                                      in_=xsrc(r0, lo, hi, W + 2))

            def load_xband(r0):
                xa = xpool.tile([128, RB + 4, W + 2], F16, tag="xa", name="xa")
                xr = xpool.tile([128, RB + 4, W + 2], F16, tag="xr", name="xr")
                emit_xchunks(xa, xr, r0, [(0, RB + 4)])
                return xa, xr

            # DMA issue order (the sync engine issues each dma_start at
            # ~665ns, so order = arrival order): the first 5 x rows unblock
            # conv1 chain 0, then the small conv1 weights/biases, then the
            # rest of band-0 x, then w2 (needed ~55us in) and the logit
            # constants (~60us).
            xband0 = (xpool.tile([128, RB + 4, W + 2], F16, tag="xa", name="xa"),
                      xpool.tile([128, RB + 4, W + 2], F16, tag="xr", name="xr"))
            emit_xchunks(*xband0, 0, [(0, 5)])

            w1rbuf = singles.tile([128, 2, 128], F16, tag="w1rbuf")
            nc.sync.dma_start(out=w1rbuf, in_=w1r.ap())
            w1r_sb = {mc: w1rbuf[:, mc, :] for mc in range(2)}
            w1sbuf = singles.tile([128, 2, 128], F16, tag="w1sbuf")
            nc.sync.dma_start(out=w1sbuf, in_=w1s.ap())
            w1s_sb = {mc: w1sbuf[:, mc, :] for mc in range(2)}

            cstbuf = singles.tile([128, 2 * K + 4], F32, tag="cstbuf")
            nc.sync.dma_start(out=cstbuf, in_=cst.ap())

            emit_xchunks(*xband0, 0, [(5, 12), (12, 24), (24, RB + 4)])
            bp_sb = cstbuf[:, 0:K]
            lab_sb = cstbuf[:, K:2 * K]
            b1_sb = {mc: cstbuf[:, 2 * K + mc:2 * K + mc + 1] for mc in range(2)}
            b2_sb = {mc: cstbuf[:, 2 * K + 2 + mc:2 * K + 3 + mc] for mc in range(2)}

            # conv2 weights are 1D-Winograd F(2,3) transformed along kh:
            # 4 positions x 2 kc x 3 dc x 2 mc slices of [128 cin, 128 out].
            w2buf = singles.tile([128, 48, 128], F16, tag="w2buf")
            nc.sync.dma_start(out=w2buf, in_=w2l.ap())
            gw_sb = {(pos, kc, dc, mc): w2buf[:, ((pos * 2 + kc) * 3 + dc) * 2 + mc, :]
                     for pos in range(4) for kc in range(2)
                     for dc in range(3) for mc in range(2)}

            muwbuf = singles.tile([128, 2, K], F16, tag="muwbuf")
            nc.sync.dma_start(out=muwbuf, in_=muw.ap())
            muw_sb = {kc: muwbuf[:, kc, :] for kc in range(2)}
            idnbuf = singles.tile([128, 128], F32, tag="idnbuf")
            nc.sync.dma_start(out=idnbuf, in_=idn.ap())

            # ---- bands ------------------------------------------------
            q1b = None
            for band in range(NBAND):
                r0 = RB * band
                # x halo in two packings:
                #  xa: p0-63 = xh rows r0..r0+19, p64-127 = same shifted +1 col
                #  xr: p0-63 = xh rows,           p64-127 = same shifted +1 row
                xa, xr = xband0 if band == 0 else load_xband(r0)

                # q1 band buffer: slot j = q1 row (r0 - 1 + j), cols 1..256
                # real, cols 0/257 replicate pads.
                q1b_prev = q1b if band > 0 else None
                q1b = {}
                for mc in range(2):
                    q1b[mc] = q1pool.tile([128, RB + 2, W + 2], F16, tag=f"q1_{mc}", name=f"q1_{mc}")

                # conv1: q1 slot j needs xh local rows j+dr (pairs), and
                # taps (0,2),(1,2) from xr row j, tap (2,2) from xa row j+2.
                if band == 0:
                    groups1 = [(j, 2) for j in range(1, RB + 1, 2)] + [(RB + 1, 1)]
                else:
                    # slots 0,1 = previous band's slots 32,33 (incl. pads):
                    # copy instead of recomputing a full conv1 chain per mc
                    for mc in range(2):
                        nc.vector.tensor_copy(
                            out=q1b[mc][:, 0:2, :],
                            in_=q1b_prev[mc][:, RB:RB + 2, :])
                    groups1 = [(j, 2) for j in range(2, RB + 2, 2)]
                for j, nr in groups1:
                    for mc in range(2):
                        c1t = ps_tile("pp_a", "c1t")
                        c1v = c1t[:, 0, 0:nr, :]
                        for dr in range(3):
                            nc.tensor.matmul(
                                c1v, w1a_sb[mc, dr],
                                xa[:, j + dr:j + dr + nr, 0:W],
                                start=(dr == 0), stop=False)
                        nc.tensor.matmul(c1v, w1r_sb[mc],
                                         xr[:, j:j + nr, 2:W + 2],
                                         start=False, stop=False)
                        # w1s is zero-padded to 128 partitions: a 64-row
                        # matmul switches the PE into half-array mode which
                        # costs ~120ns extra turnaround per chain.
                        nc.tensor.matmul(c1v, w1s_sb[mc],
                                         xa[:, j + 2:j + 2 + nr, 2:W + 2],
                                         start=False, stop=True)
                        nc.scalar.activation(
                            out=q1b[mc][:, j:j + nr, 1:W + 1], in_=c1v,
                            func=AF.Relu, bias=b1_sb[mc], scale=1.0)
                        # replicate col pads per chain (not once per band)
                        # so the first Winograd transforms don't wait on the
                        # whole conv1 ACT chain.
                        nc.vector.tensor_copy(
                            out=q1b[mc][:, j:j + nr, 0:1],
                            in_=q1b[mc][:, j:j + nr, 1:2])
                        nc.vector.tensor_copy(
                            out=q1b[mc][:, j:j + nr, W + 1:W + 2],
                            in_=q1b[mc][:, j:j + nr, W:W + 1])
                        if band == 0 and j == 1:
                            # image-top replicate row (incl. its col pads)
                            nc.vector.tensor_copy(
                                out=q1b[mc][:, 0:1, :], in_=q1b[mc][:, 1:2, :])

                ob = obuf.tile([128, RB // 2, 4], F32, tag="ob", name="ob")

                def logits_softmax(q2t, gp):
                    # logits for BOTH groups of pair gp into one psum tile
                    # ([128 px, 2*4*K] fits a half bank), q2 stationary; one
                    # fused softmax chain over both groups.
                    plt = ps_tile("pp_b", "plt")
                    plv = plt[:, 0, 0, 0:2 * 4 * K].rearrange(
                        "p (gg j k) -> p gg j k", gg=2, j=4)
                    for gg in range(2):
                        for j in range(4):
                            for kc in range(2):
                                q2flat = q2t[(kc, gg)].rearrange("p a b -> p (a b)")
                                nc.tensor.matmul(
                                    plv[:, gg, j, :],
                                    q2flat[:, 128 * j:128 * (j + 1)],
                                    muw_sb[kc], start=(kc == 0), stop=(kc == 1))
                    pl = plv.rearrange("p gg j k -> p (gg j) k")
                    # softmax over K (free axis) + label contraction
                    li = smx.tile([128, 8, K], F32, tag="li", name="li")
                    nc.vector.tensor_tensor(
                        li, pl,
                        bp_sb.unsqueeze(1).to_broadcast([128, 8, K]),
                        ALU.add)
                    mx = smx.tile([128, 8], F32, tag="mx", name="mx")
                    nc.vector.reduce_max(mx, li, axis=AX.X)
                    ls = smx.tile([128, 8, K], F32, tag="ls", name="ls")
                    nc.vector.tensor_tensor(
                        ls, li,
                        mx.unsqueeze(2).to_broadcast([128, 8, K]),
                        ALU.subtract)
                    ex = smx.tile([128, 8, K], F32, tag="ex", name="ex")
                    nc.scalar.activation(out=ex, in_=ls, func=AF.Exp)
                    el = smx.tile([128, 8, K], F32, tag="el", name="el")
                    nc.vector.tensor_tensor(
                        el, ex,
                        lab_sb.unsqueeze(1).to_broadcast([128, 8, K]),
                        ALU.mult)
                    ssum = smx.tile([128, 8], F32, tag="ssum", name="ssum")
                    nc.vector.reduce_sum(ssum, ex, axis=AX.X)
                    wsum = smx.tile([128, 8], F32, tag="wsum", name="wsum")
                    nc.vector.reduce_sum(wsum, el, axis=AX.X)
                    rs = smx.tile([128, 8], F32, tag="rs", name="rs")
                    nc.vector.reciprocal(rs, ssum)
                    nc.vector.tensor_tensor(
                        ob[:, 2 * gp:2 * gp + 2],
                        wsum.rearrange("p (gg j) -> p gg j", gg=2),
                        rs.rearrange("p (gg j) -> p gg j", gg=2), ALU.mult)

                pending = []
                for gp in range(RB // 4):
                    # conv2 via 1D-Winograd F(2,3) over rows, TWO groups
                    # (4 output rows) per pass so each matmul runs at free
                    # dim 512: halves the LDWEIGHTS count, which otherwise
                    # bounds the PE sequencer at 2 instructions per 107ns.
                    # Forward transform (vector+gpsimd, fp16):
                    #   t0 = d0-d2; t1 = d1+d2; t2 = d2-d1; t3 = d1-d3
                    tb = {}
                    for kc in range(2):
                        tb[kc] = tpool.tile([128, 4, 2, W + 2], F16,
                                            tag=f"tb{kc}", name=f"tb{kc}")
                        q = q1b[kc]
                        for gg in range(2):
                            j = 2 * (2 * gp + gg)
                            nc.vector.tensor_tensor(
                                tb[kc][:, 0, gg], q[:, j], q[:, j + 2],
                                ALU.subtract)
                            nc.vector.tensor_tensor(
                                tb[kc][:, 1, gg], q[:, j + 1], q[:, j + 2],
                                ALU.add)
                            # t2 on GpSimd (slow engine: ~750ns/op, so only
                            # a quarter of the transform goes there)
                            nc.gpsimd.tensor_tensor(
                                tb[kc][:, 2, gg], q[:, j + 2], q[:, j + 1],
                                ALU.subtract)
                            nc.vector.tensor_tensor(
                                tb[kc][:, 3, gg], q[:, j + 1], q[:, j + 3],
                                ALU.subtract)
                    # Position GEMMs: Yp = sum_{kc,dc} gw[p,kc,dc] @ t[kc][p]
                    # for both groups at once; two positions per psum tag.
                    # Inverse A^T: y0 = Y0+Y1+Y2, y1 = Y1-Y2-Y3 (vector;
                    # Y1 staged through SBUF by a scalar copy since vector
                    # ops may read only one PSUM operand) + bias/relu.
                    q2t = {}
                    for mc in range(2):
                        pa = ps_tile("pp_a", "pa")
                        pb = ps_tile("pp_b", "pb")
                        for pos in range(4):
                            pt = (pa, pb)[pos // 2][:, pos % 2]
                            n_mm = 0
                            for kc in range(2):
                                for dc in range(3):
                                    nc.tensor.matmul(
                                        pt, gw_sb[pos, kc, dc, mc],
                                        tb[kc][:, pos, :, dc:dc + W],
                                        start=(n_mm == 0), stop=(n_mm == 5))
                                    n_mm += 1
                        for gg in range(2):
                            s1 = smx.tile([128, W], F32, tag="s1", name="s1")
                            nc.scalar.activation(out=s1, in_=pa[:, 1, gg, :],
                                                 func=AF.Copy, scale=1.0)
                            vu = smx.tile([128, W], F32, tag="vu", name="vu")
                            nc.vector.tensor_tensor(vu, s1, pa[:, 0, gg, :],
                                                    ALU.add)
                            v0 = smx.tile([128, W], F32, tag="v0", name="v0")
                            nc.vector.tensor_tensor(v0, vu, pb[:, 0, gg, :],
                                                    ALU.add)
                            vw = smx.tile([128, W], F32, tag="vw", name="vw")
                            nc.vector.tensor_tensor(vw, s1, pb[:, 0, gg, :],
                                                    ALU.subtract)
                            v1 = smx.tile([128, W], F32, tag="v1", name="v1")
                            nc.vector.tensor_tensor(v1, vw, pb[:, 1, gg, :],
                                                    ALU.subtract)
                            qt = q2pool.tile([128, 2, W], F16,
                                             tag=f"q2_{mc}_{gg}",
                                             name=f"q2_{mc}_{gg}")
                            nc.scalar.activation(out=qt[:, 0, :], in_=v0,
                                                 func=AF.Relu, bias=b2_sb[mc],
                                                 scale=1.0)
                            nc.scalar.activation(out=qt[:, 1, :], in_=v1,
                                                 func=AF.Relu, bias=b2_sb[mc],
                                                 scale=1.0)
                            q2t[(mc, gg)] = qt
                    # software pipeline: the previous pair's logits run here,
                    # after this pair's GEMMs are queued, so the PE never
                    # waits on this pair's inverse+relu chain.
                    for prev_q2t, prev_gp in pending:
                        logits_softmax(prev_q2t, prev_gp)
                    pending = [(q2t, gp)]
                for prev_q2t, prev_gp in pending:
                    logits_softmax(prev_q2t, prev_gp)
                pending = []

                # ob[p, g, (r jj)] -> PE-transpose to row-major [32, 256] so
                # the dram write is 32 contiguous 1KB rows instead of a
                # 4-byte-element scatter (which cost ~24us exposed at the
                # kernel tail).
                obr = ob.rearrange("p g (r jj) -> p g r jj", r=2)
                ot = obuf.tile([RB, W], F32, tag="ot", name="ot")
                for jj in range(2):
                    tpt = ps_tile("pp_b", "tpt")
                    tps = tpt[0:RB, 0, 0, 0:128]
                    nc.tensor.transpose(tps, obr[:, :, :, jj], idnbuf)
                    nc.vector.tensor_copy(out=ot[:, 128 * jj:128 * (jj + 1)],
                                          in_=tps)
                nc.sync.dma_start(out=outd.ap()[r0:r0 + RB, :], in_=ot)

    nc.compile()
    return nc


def prep_inputs(x, w1, b1, w2, b2, w3, b3, mu, label):
    """Full inputs -> per-core in_maps."""
    w3m = w3[:, :, 0, 0]
    muW = 2.0 * (mu @ w3m)                                   # [K, Q]
    bpv = (2.0 * (mu @ b3) - (mu * mu).sum(1)).astype(np.float32)

    def pack_w(w1f, w2f):
        cinw = w1f.shape[1]
        w1a = np.empty((2, 3, 128, 128), np.float32)
        w1r = np.empty((2, 128, 128), np.float32)
        w1s = np.zeros((2, 128, 128), np.float32)
        for mc in range(2):
            ms = slice(128 * mc, 128 * (mc + 1))
            for dr in range(3):
                w1a[mc, dr, 0:64] = w1f[ms, :, dr, 0].T
                w1a[mc, dr, 64:128] = w1f[ms, :, dr, 1].T
            w1r[mc, 0:64] = w1f[ms, :, 0, 2].T
            w1r[mc, 64:128] = w1f[ms, :, 1, 2].T
            w1s[mc, 0:64] = w1f[ms, :, 2, 2].T
        # 1D-Winograd F(2,3) transform of w2 along kh:
        #   g0 = w[0]; g1 = (w[0]+w[1]+w[2])/2; g2 = (w[0]-w[1]+w[2])/2; g3 = w[2]
        w0, wk1, wk2 = w2f[:, :, 0, :], w2f[:, :, 1, :], w2f[:, :, 2, :]
        gws = [w0, 0.5 * (w0 + wk1 + wk2), 0.5 * (w0 - wk1 + wk2), wk2]
        w2p = np.empty((48, 128, 128), np.float32)
        for pos in range(4):
            for kc in range(2):
                for dc in range(3):
                    for mc in range(2):
                        idx = ((pos * 2 + kc) * 3 + dc) * 2 + mc
                        w2p[idx] = gws[pos][128 * mc:128 * (mc + 1),
                                            128 * kc:128 * (kc + 1), dc].T
        return (np.ascontiguousarray(w1a.reshape(6, 128, 128).transpose(1, 0, 2)).astype(np.float16),
                np.ascontiguousarray(w1r.transpose(1, 0, 2)).astype(np.float16),
                np.ascontiguousarray(w1s.transpose(1, 0, 2)).astype(np.float16),
                np.ascontiguousarray(w2p.transpose(1, 0, 2)).astype(np.float16))

    packs = {}
    packs[0] = pack_w(w1, w2)
    packs[1] = pack_w(w1[:, :, ::-1, :], w2[:, :, ::-1, :])

    muwp = np.empty((128, 2, K), np.float32)
    for kc in range(2):
        muwp[:, kc, :] = muW[:, 128 * kc:128 * (kc + 1)].T
    muwp = muwp.astype(np.float16)
    cstv = np.empty((128, 2 * K + 4), np.float32)
    cstv[:, 0:K] = bpv[None, :]
    cstv[:, K:2 * K] = label[None, :].astype(np.float32)
    for mc in range(2):
        cstv[:, 2 * K + mc] = b1[128 * mc:128 * (mc + 1)]
        cstv[:, 2 * K + 2 + mc] = b2[128 * mc:128 * (mc + 1)]

    rows = np.clip(np.arange(132) - 2, 0, H - 1)
    cols = np.clip(np.arange(W + 2) - 1, 0, W - 1)
    in_maps = []
    for core in range(NCORES):
        img, half = core // 2, core % 2
        xl = x[img] if half == 0 else x[img, :, ::-1, :]
        xhv = np.ascontiguousarray(xl[:, rows][:, :, cols]).astype(np.float16)
        xhf = np.concatenate([xhv.reshape(-1),
                              np.zeros(2 * (W + 2), np.float16)])
        w1ap, w1rp, w1sp, w2p = packs[half]
        in_maps.append({
            'xh': xhf, 'w1a': w1ap, 'w1r': w1rp, 'w1s': w1sp, 'w2l': w2p,
            'muw': muwp, 'cst': cstv, 'idn': np.eye(128, dtype=np.float32),
        })
    return in_maps


def gather(results, dtype=np.float32):
    out = np.empty((B, 1, H, W), dtype)
    for core in range(NCORES):
        img, half = core // 2, core % 2
        o = results[core]['out']
        if half == 0:
            out[img, 0, 0:128] = o
        else:
            out[img, 0, 128:256] = o[::-1]
    return out


def get_nc():
    if 'nc' not in _cached:
        _cached['nc'] = build_nc()
    return _cached['nc']


def kernel(x, w1, b1, w2, b2, w3, b3, mu, label, **run_kwargs):
    nc = get_nc()
    in_maps = prep_inputs(
        np.asarray(x, np.float32), np.asarray(w1, np.float32),
        np.asarray(b1, np.float32), np.asarray(w2, np.float32),
        np.asarray(b2, np.float32), np.asarray(w3, np.float32),
        np.asarray(b3, np.float32), np.asarray(mu, np.float32),
        np.asarray(label, np.float32))
    res = run_bass_kernel_spmd(nc, in_maps, core_ids=list(range(NCORES)),
                               **run_kwargs)
    out = gather(res.results)
    if run_kwargs:
        _cached['last_result'] = res
    return out

